# revision 5
# baseline (speedup 1.0000x reference)
"""CPC_Net forward pass: StagerNet embeddings on 8 NeuronCores (Bass),
GRU + bilinear scoring on host.

Device kernel layout: activations live as [(b4, c, o) partitions, position free].
conv2: block-diag over b-quads, K=(kp16,b4,ci2)=128, 4 accumulated tap-chunks,
       M=(b4,c,o)=128 Toeplitz weight columns (w1 folded in).
conv3: block-diag over s=(b4,c) blocks, 50 accumulated taps, tap = rhs free-offset.
BN: relu+sum via ACT accum_out, sumsq via DVE scalar_tensor_tensor accum,
    per-channel aggregation via mask matmuls, affine applied post-pool (g>0).
"""
import os
import sys

sys.path.insert(0, "/opt/trn_rl_repo")

import numpy as np

EPS = 1e-5
B, Np, Nb, T, C, H = 16, 16, 10, 3000, 2, 100
O = 16
L1 = T - 49            # 2951 = 13*227
G1 = L1 // 13          # 227
L2 = G1 - 49           # 178
G2 = 13
TP = 3072              # padded input length
NQ = 4                 # b-quads per window
KPC = 16               # kp positions per conv2 tap-chunk
NT2 = 4                # conv2 tap chunks
PC1 = 1024             # conv2 psum/relu chunk
W_PER_CORE = 24
N_CORES = 8

LAST_EXEC_NS = 0


# ---------------------------------------------------------------------------
# toolchain patches: this sandbox's walrus accepts at most ONE sync wait per
# instruction. Split multi-wait instructions at BIR-JSON level; split the
# Tile tail drain's waits across NoOps.
def _install_patches():
    import json as _json
    import concourse.bass as _bass
    import concourse.tile as _tile
    from concourse.vector_clock import ScopedClock
    import bass_rust

    if getattr(_bass.Bass, "_cpc_patched", False):
        class _Holder:
            pass
    else:
        _orig = _bass.Bass.to_json_bytes

        def _split_multiwait_json(self):
            raw = _orig(self)
            data = _json.loads(raw)
            changed = False
            for fn in data.get("functions", []):
                for blk in fn.get("blocks", []):
                    insts = blk.get("instructions", [])
                    out = []
                    for inst in insts:
                        si = inst.get("sync_info")
                        waits = (si or {}).get("on_wait") or []
                        if len(waits) > 1:
                            changed = True
                            for k, wx in enumerate(waits[:-1]):
                                out.append({
                                    "debug": inst.get("debug", 0),
                                    "engine": inst["engine"],
                                    "ins": [], "outs": [],
                                    "name": f"{inst['name']}-sw{k}",
                                    "opcode": "NoOp",
                                    "sync_info": {"on_update": [], "on_wait": [wx]},
                                    "text_hint": "split_wait",
                                })
                            si["on_wait"] = waits[-1:]
                        out.append(inst)
                    if changed:
                        blk["instructions"] = out
            return _json.dumps(data).encode() if changed else raw

        _bass.Bass.to_json_bytes = _split_multiwait_json
        _bass.Bass._cpc_patched = True

    class PatchedTC(_tile.TileContext):
        def _drain_and_barrier(self, tick_clock, wait_clock):
            probe = self.nc.sync.nop(hint="drain_waits", nofuse=True)
            wait_clock.add_sem_waits(
                probe.ins, ScopedClock({None: tick_clock.global_clock})
            )
            si = probe.ins.sync_info
            waits = list(si.on_wait) if si is not None else []
            if len(waits) > 1:
                si.on_wait = waits[:1]
                for w in waits[1:]:
                    n2 = self.nc.sync.nop(hint="drain_waits2", nofuse=True)
                    s2 = n2.ins.sync_info
                    if s2 is None:
                        n2.ins.sync_info = bass_rust.SyncInfo(on_wait=[w], on_update=[])
                    else:
                        s2.on_wait = [w]
            self.nc.sync.drain()
            self.nc.all_engine_barrier()
            popped = self.nc._tile_sem_poison_stack.pop()
            assert popped is self._sem_poison
            self.nc.clear_and_free_semaphores(list(self.sems.allocated().values()))
            self.nc.all_engine_barrier()

    return PatchedTC


# ---------------------------------------------------------------------------
def _host_weights(w1, w2, w3):
    """Precompute device weight matrices. w1:(C,C) w2:(16,50) w3:(16,16,50)."""
    cw2 = np.zeros((NT2, 128, 128), np.float32)
    for t in range(NT2):
        for kp in range(KPC):
            r = t * KPC + kp
            if r >= 50:
                continue
            for b4 in range(4):
                p = kp * 8 + b4 * 2
                for ci in range(C):
                    for c in range(C):
                        m0 = b4 * 32 + c * 16
                        cw2[t, p + ci, m0:m0 + O] = w2[:, r] * w1[c, ci]
    cw3 = np.zeros((50, 128, 128), np.float32)
    for r in range(50):
        blk = w3[:, :, r].T
        for s in range(8):
            cw3[r, s * 16:(s + 1) * 16, s * 16:(s + 1) * 16] = blk
    mask = np.zeros((128, 16), np.float32)
    for p in range(128):
        mask[p, p % 16] = 1.0
    return cw2, cw3, mask, np.ascontiguousarray(mask.T)


def _build_kernel(W):
    import concourse.bass as bass
    import concourse.mybir as mybir

    PatchedTC = _install_patches()
    f32 = mybir.dt.float32
    bf16 = mybir.dt.bfloat16
    AF = mybir.ActivationFunctionType
    OP = mybir.AluOpType

    nc = bass.Bass(num_devices=N_CORES, debug=False)
    xw_d = nc.dram_tensor("xw", [W, B, C, TP], f32, kind="ExternalInput")
    cw2_d = nc.dram_tensor("cw2", [NT2, 128, 128], f32, kind="ExternalInput")
    cw3_d = nc.dram_tensor("cw3", [50, 128, 128], f32, kind="ExternalInput")
    mask_d = nc.dram_tensor("mask", [128, 16], f32, kind="ExternalInput")
    maskT_d = nc.dram_tensor("maskT", [16, 128], f32, kind="ExternalInput")
    bnp_d = nc.dram_tensor("bnp", [16, 4], f32, kind="ExternalInput")
    x4_d = nc.dram_tensor("x4", [W, NQ, 128, G2], f32, kind="ExternalOutput")

    N1 = float(B * L1 * C)
    N2 = float(B * L2 * C)

    with PatchedTC(nc) as tc:
        with (
            tc.tile_pool(name="const", bufs=1) as cp,
            tc.tile_pool(name="xin", bufs=3) as xp,
            tc.tile_pool(name="act", bufs=2) as ap,
            tc.tile_pool(name="scr", bufs=1) as scp,
            tc.tile_pool(name="zp", bufs=2) as zpp,
            tc.tile_pool(name="stat", bufs=2) as stp,
            tc.tile_pool(name="ps", bufs=2, space="PSUM") as psp,
            tc.tile_pool(name="ps3", bufs=2, space="PSUM") as ps3p,
            tc.tile_pool(name="pss", bufs=2, space="PSUM") as pssp,
        ):
            cw2_t = cp.tile([128, NT2 * 128], f32)
            for t in range(NT2):
                nc.sync.dma_start(out=cw2_t[:, t * 128:(t + 1) * 128], in_=cw2_d[t])
            cw3_t = cp.tile([128, 50 * 128], f32)
            for r in range(50):
                nc.sync.dma_start(out=cw3_t[:, r * 128:(r + 1) * 128], in_=cw3_d[r])
            mask_t = cp.tile([128, 16], f32)
            nc.sync.dma_start(out=mask_t[:, :], in_=mask_d[:, :])
            maskT_t = cp.tile([16, 128], f32)
            nc.sync.dma_start(out=maskT_t[:, :], in_=maskT_d[:, :])
            bnp_t = cp.tile([16, 4], f32)
            nc.sync.dma_start(out=bnp_t[:, :], in_=bnp_d[:, :])

            for w in range(W):
                zp_tiles = []
                st1 = stp.tile([128, 24], f32, tag="st1")
                for q in range(NQ):
                    x2s = xp.tile([128, T], f32, tag="x2s")
                    for kp in range(KPC):
                        nc.sync.dma_start(
                            out=x2s[kp * 8:(kp + 1) * 8, :],
                            in_=xw_d[w, q * 4:(q + 1) * 4, :, kp:kp + T].rearrange("b c t -> (b c) t"),
                        )
                    zr1 = ap.tile([128, L1], f32, tag="zr1")
                    scr = scp.tile([128, PC1], f32, tag="scr")
                    for pc in range(3):
                        p0 = pc * PC1
                        ncol = min(PC1, L1 - p0)
                        zc = psp.tile([128, PC1], f32, tag="zc")
                        for sub in range(0, ncol, 512):
                            nsub = min(512, ncol - sub)
                            for t in range(NT2):
                                nc.tensor.matmul(
                                    zc[:, sub:sub + nsub],
                                    cw2_t[:, t * 128:(t + 1) * 128],
                                    x2s[:, p0 + sub + t * KPC: p0 + sub + t * KPC + nsub],
                                    start=(t == 0), stop=(t == NT2 - 1),
                                )
                        nc.scalar.activation(
                            out=zr1[:, p0:p0 + ncol], in_=zc[:, :ncol], func=AF.Relu,
                            accum_out=st1[:, q * 3 + pc: q * 3 + pc + 1],
                        )
                        nc.vector.scalar_tensor_tensor(
                            out=scr[:, :ncol], in0=zr1[:, p0:p0 + ncol], scalar=1.0,
                            in1=zr1[:, p0:p0 + ncol], op0=OP.mult, op1=OP.mult,
                            accum_out=st1[:, 12 + q * 3 + pc: 12 + q * 3 + pc + 1],
                        )
                    zp1 = zpp.tile([128, G1], f32, tag=f"zp{q}")
                    nc.vector.tensor_reduce(
                        out=zp1[:, :], in_=zr1[:, :].rearrange("p (g j) -> p g j", j=13),
                        axis=mybir.AxisListType.X, op=OP.max,
                    )
                    zp_tiles.append(zp1)

                ms1 = pssp.tile([128, 32], f32, tag="msx")
                nc.tensor.matmul(ms1[0:16, 0:24], mask_t[:, :], st1[:, :], start=True, stop=True)
                s1 = stp.tile([16, 2], f32, tag="s1")
                nc.vector.tensor_reduce(out=s1[:, 0:1], in_=ms1[0:16, 0:12],
                                        axis=mybir.AxisListType.X, op=OP.add)
                nc.vector.tensor_reduce(out=s1[:, 1:2], in_=ms1[0:16, 12:24],
                                        axis=mybir.AxisListType.X, op=OP.add)
                fin1 = stp.tile([16, 6], f32, tag="fin1")
                nc.vector.tensor_scalar_mul(fin1[:, 0:1], s1[:, 0:1], 1.0 / N1)
                nc.vector.tensor_scalar_mul(fin1[:, 1:2], s1[:, 1:2], 1.0 / N1)
                nc.vector.tensor_mul(fin1[:, 2:3], fin1[:, 0:1], fin1[:, 0:1])
                nc.vector.tensor_sub(fin1[:, 2:3], fin1[:, 1:2], fin1[:, 2:3])
                nc.vector.tensor_scalar_add(fin1[:, 2:3], fin1[:, 2:3], EPS)
                nc.vector.reciprocal(fin1[:, 3:4], fin1[:, 2:3])
                nc.scalar.activation(out=fin1[:, 3:4], in_=fin1[:, 3:4], func=AF.Sqrt)
                nc.vector.tensor_mul(fin1[:, 4:5], bnp_t[:, 0:1], fin1[:, 3:4])
                nc.vector.tensor_mul(fin1[:, 5:6], fin1[:, 0:1], fin1[:, 4:5])
                nc.vector.tensor_sub(fin1[:, 5:6], bnp_t[:, 1:2], fin1[:, 5:6])
                bc1 = pssp.tile([128, 32], f32, tag="msx")
                nc.tensor.matmul(bc1[:, 0:2], maskT_t[:, :], fin1[:, 4:6], start=True, stop=True)
                scb1 = stp.tile([128, 2], f32, tag="scb1")
                nc.vector.tensor_copy(scb1[:, :], bc1[:, 0:2])

                x3all = ap.tile([128, NQ * 232], f32, tag="x3all")
                for q in range(NQ):
                    nc.vector.tensor_scalar(
                        out=x3all[:, q * 232:q * 232 + G1], in0=zp_tiles[q][:, :],
                        scalar1=scb1[:, 0:1], scalar2=scb1[:, 1:2], op0=OP.mult, op1=OP.add,
                    )
                st2 = stp.tile([128, 8], f32, tag="st2")
                zr3_tiles = []
                for q in range(NQ):
                    c3 = ps3p.tile([128, L2], f32, tag="c3")
                    for r in range(50):
                        nc.tensor.matmul(
                            c3[:, :],
                            cw3_t[:, r * 128:(r + 1) * 128],
                            x3all[:, q * 232 + r: q * 232 + r + L2],
                            start=(r == 0), stop=(r == 49),
                        )
                    zr3 = ap.tile([128, L2], f32, tag=f"zr3_{q}")
                    nc.scalar.activation(
                        out=zr3[:, :], in_=c3[:, :], func=AF.Relu,
                        accum_out=st2[:, q: q + 1],
                    )
                    scr3 = scp.tile([128, PC1], f32, tag="scr")
                    nc.vector.scalar_tensor_tensor(
                        out=scr3[:, :L2], in0=zr3[:, :], scalar=1.0, in1=zr3[:, :],
                        op0=OP.mult, op1=OP.mult,
                        accum_out=st2[:, 4 + q: 5 + q],
                    )
                    zr3_tiles.append(zr3)
                ms2 = pssp.tile([128, 32], f32, tag="msx")
                nc.tensor.matmul(ms2[0:16, 0:8], mask_t[:, :], st2[:, :], start=True, stop=True)
                s2 = stp.tile([16, 2], f32, tag="s2")
                nc.vector.tensor_reduce(out=s2[:, 0:1], in_=ms2[0:16, 0:4],
                                        axis=mybir.AxisListType.X, op=OP.add)
                nc.vector.tensor_reduce(out=s2[:, 1:2], in_=ms2[0:16, 4:8],
                                        axis=mybir.AxisListType.X, op=OP.add)
                fin2 = stp.tile([16, 6], f32, tag="fin2")
                nc.vector.tensor_scalar_mul(fin2[:, 0:1], s2[:, 0:1], 1.0 / N2)
                nc.vector.tensor_scalar_mul(fin2[:, 1:2], s2[:, 1:2], 1.0 / N2)
                nc.vector.tensor_mul(fin2[:, 2:3], fin2[:, 0:1], fin2[:, 0:1])
                nc.vector.tensor_sub(fin2[:, 2:3], fin2[:, 1:2], fin2[:, 2:3])
                nc.vector.tensor_scalar_add(fin2[:, 2:3], fin2[:, 2:3], EPS)
                nc.vector.reciprocal(fin2[:, 3:4], fin2[:, 2:3])
                nc.scalar.activation(out=fin2[:, 3:4], in_=fin2[:, 3:4], func=AF.Sqrt)
                nc.vector.tensor_mul(fin2[:, 4:5], bnp_t[:, 2:3], fin2[:, 3:4])
                nc.vector.tensor_mul(fin2[:, 5:6], fin2[:, 0:1], fin2[:, 4:5])
                nc.vector.tensor_sub(fin2[:, 5:6], bnp_t[:, 3:4], fin2[:, 5:6])
                bc2 = pssp.tile([128, 32], f32, tag="msx")
                nc.tensor.matmul(bc2[:, 0:2], maskT_t[:, :], fin2[:, 4:6], start=True, stop=True)
                scb2 = stp.tile([128, 2], f32, tag="scb2")
                nc.vector.tensor_copy(scb2[:, :], bc2[:, 0:2])

                for q in range(NQ):
                    zp3 = zpp.tile([128, G2], f32, tag="zp3")
                    nc.vector.tensor_reduce(
                        out=zp3[:, :], in_=zr3_tiles[q][:, 0:169].rearrange("p (g j) -> p g j", j=13),
                        axis=mybir.AxisListType.X, op=OP.max,
                    )
                    x4q = zpp.tile([128, G2], f32, tag="x4q")
                    nc.vector.tensor_scalar(
                        out=x4q[:, :], in0=zp3[:, :],
                        scalar1=scb2[:, 0:1], scalar2=scb2[:, 1:2], op0=OP.mult, op1=OP.add,
                    )
                    nc.sync.dma_start(out=x4_d[w, q], in_=x4q[:, :])
    return nc


def _device_stagenet(Xc, Xp, Xb, w1, w2, w3, g1, be1, g2, be2):
    """-> (192, B, 416) pooled features in window order [Xb(160), Xc(16), Xp(16)]."""
    global LAST_EXEC_NS
    from concourse import bass_utils

    # pack windows: (192, B, C, TP)
    xb = Xb.transpose(1, 2, 0, 4, 3).reshape(Np * Nb, B, C, T)
    xc = Xc.transpose(1, 0, 3, 2)
    xpp = Xp.transpose(1, 0, 3, 2)
    allw = np.concatenate([xb, xc, xpp], axis=0)
    xw_all = np.zeros((192, B, C, TP), np.float32)
    xw_all[:, :, :, :T] = allw

    cw2, cw3, mask, maskT = _host_weights(w1, w2, w3)
    bnp = np.stack([g1, be1, g2, be2], axis=1).astype(np.float32)

    W = W_PER_CORE
    nc = _build_kernel(W)
    in_maps = []
    for k in range(N_CORES):
        in_maps.append({
            "xw": np.ascontiguousarray(xw_all[k * W:(k + 1) * W]),
            "cw2": cw2, "cw3": cw3, "mask": mask, "maskT": maskT, "bnp": bnp,
        })
    import time as _time
    _t0 = _time.time()
    res = bass_utils.run_bass_kernel_spmd(nc, in_maps, core_ids=list(range(N_CORES)))
    LAST_EXEC_NS = int((_time.time() - _t0) * 1e9)

    feats = np.empty((192, B, 16 * G2 * C), np.float32)
    for k in range(N_CORES):
        x4 = res.results[k]["x4"]                    # (W, 4, 128, G2)
        x = x4.reshape(W, 4, 4, 2, 16, G2)           # (w, quad, b4, c, o3, g)
        f = x.transpose(0, 1, 2, 4, 5, 3)            # (w, quad, b4, o3, g, c)
        feats[k * W:(k + 1) * W] = f.reshape(W, 16, 16 * G2 * 2)
    return feats


# ---------------------------------------------------------------------------
# host fallback stagenet (numpy), from the original baseline
def _conv_tap(x, K):
    S, L = x.shape
    O_, R = K.shape
    Lo = L - R + 1
    out = np.empty((S, O_, Lo), np.float32)
    sv = np.lib.stride_tricks.as_strided(
        x, (S, Lo, R), (x.strides[0], x.strides[1], x.strides[1])
    )
    cs = max(1, S // 8)
    for i in range(0, S, cs):
        out[i:i + cs] = np.tensordot(sv[i:i + cs], K, axes=([2], [1])).transpose(0, 2, 1)
    return out


def _stagenet_batch_host(x, p):
    N = x.shape[0]
    y = x @ p["w1"].T + p["b1"]
    seq = y.transpose(0, 1, 3, 2).reshape(N * B * C, T).astype(np.float32)
    z = _conv_tap(seq, p["w2"])
    z = z.reshape(N, B, C, 16, L1).transpose(0, 1, 3, 4, 2) + p["b2"][None, None, :, None, None]
    z = np.maximum(z, 0.0)
    m = z.mean(axis=(1, 3, 4), keepdims=True)
    v = z.var(axis=(1, 3, 4), keepdims=True)
    z = (z - m) / np.sqrt(v + EPS) * p["g1"][None, None, :, None, None] + p["be1"][None, None, :, None, None]
    z = z.reshape(N, B, 16, G1, 13, C).max(axis=4)
    seq3 = z.transpose(0, 1, 4, 2, 3).reshape(N * B * C, 16, G1)
    sv = np.lib.stride_tricks.as_strided(
        seq3, (N * B * C, L2, 16, 50),
        (seq3.strides[0], seq3.strides[2], seq3.strides[1], seq3.strides[2]),
    )
    W3 = p["w3"].reshape(16, 16 * 50)
    z3 = np.empty((N * B * C, L2, 16), np.float32)
    cs = max(1, (N * B * C) // 8)
    for i in range(0, N * B * C, cs):
        z3[i:i + cs] = (sv[i:i + cs].reshape(-1, 16 * 50) @ W3.T).reshape(-1, L2, 16)
    z3 = z3.reshape(N, B, C, L2, 16).transpose(0, 1, 4, 3, 2) + p["b3"][None, None, :, None, None]
    z3 = np.maximum(z3, 0.0)
    m = z3.mean(axis=(1, 3, 4), keepdims=True)
    v = z3.var(axis=(1, 3, 4), keepdims=True)
    z3 = (z3 - m) / np.sqrt(v + EPS) * p["g2"][None, None, :, None, None] + p["be2"][None, None, :, None, None]
    z3 = z3[:, :, :, :169, :].reshape(N, B, 16, 13, 13, C).max(axis=4)
    return z3.reshape(N, B, 16 * 13 * C)


def _gru_last(xs, wih, whh, bih, bhh):
    h = np.zeros((xs.shape[0], whh.shape[1]), np.float32)
    for t in range(xs.shape[1]):
        gi = xs[:, t] @ wih.T + bih
        gh = h @ whh.T + bhh
        ir, iz, inn = np.split(gi, 3, axis=-1)
        hr, hz, hnn = np.split(gh, 3, axis=-1)
        r = 1.0 / (1.0 + np.exp(-(ir + hr)))
        z = 1.0 / (1.0 + np.exp(-(iz + hz)))
        n = np.tanh(inn + r * hnn)
        h = (1.0 - z) * n + z * h
    return h


def kernel(Xc, Xp, Xb, sn_w1, sn_b1, sn_w2, sn_b2, sn_g1, sn_be1, sn_w3, sn_b3,
           sn_g2, sn_be2, sn_wl, sn_bl, gru_wih, gru_whh, gru_bih, gru_bhh, bilin_w):
    Xc = np.asarray(Xc, np.float32)
    Xp = np.asarray(Xp, np.float32)
    Xb = np.asarray(Xb, np.float32)
    w1 = np.asarray(sn_w1, np.float32)
    w2 = np.asarray(sn_w2, np.float32).reshape(16, 50)
    w3 = np.asarray(sn_w3, np.float32)[:, :, :, 0]
    g1 = np.asarray(sn_g1, np.float32); be1 = np.asarray(sn_be1, np.float32)
    g2 = np.asarray(sn_g2, np.float32); be2 = np.asarray(sn_be2, np.float32)
    wl = np.asarray(sn_wl, np.float32); bl = np.asarray(sn_bl, np.float32)
    b1 = np.asarray(sn_b1, np.float32); b2 = np.asarray(sn_b2, np.float32)
    b3 = np.asarray(sn_b3, np.float32)

    use_device = (
        abs(b1).max() == 0 and abs(b2).max() == 0 and abs(b3).max() == 0
        and (g1 > 0).all() and (g2 > 0).all()
    )
    feats = None
    if use_device:
        try:
            feats = _device_stagenet(Xc, Xp, Xb, w1, w2, w3, g1, be1, g2, be2)
        except Exception as e:
            print(f"kernel: device stagenet failed ({type(e).__name__}: {e}); host fallback")
            feats = None
    if feats is None:
        p = {"w1": w1, "b1": b1, "w2": w2, "b2": b2, "g1": g1, "be1": be1,
             "w3": w3, "b3": b3, "g2": g2, "be2": be2}
        xb_in = Xb.transpose(1, 2, 0, 3, 4).reshape(Np * Nb, B, T, C)
        xc_in = Xc.transpose(1, 0, 2, 3)
        xp_in = Xp.transpose(1, 0, 2, 3)
        feats = np.concatenate([
            _stagenet_batch_host(xb_in, p),
            _stagenet_batch_host(xc_in, p),
            _stagenet_batch_host(xp_in, p),
        ], axis=0)

    emb = feats @ wl.T + bl                       # (192, B, 100)
    xb_e = emb[:160].reshape(Np, Nb, B, H).transpose(2, 0, 1, 3)   # (B,Np,Nb,H)
    xc_e = emb[160:176].transpose(1, 0, 2)        # (B,16,H)
    xp_e = emb[176:192].transpose(1, 0, 2)        # (B,Np,H)

    hn = _gru_last(xc_e, np.asarray(gru_wih, np.float32), np.asarray(gru_whh, np.float32),
                   np.asarray(gru_bih, np.float32), np.asarray(gru_bhh, np.float32))
    cand = np.concatenate([xp_e[:, :, None, :], xb_e], axis=2)     # (B,Np,Nb+1,H)
    out = np.einsum("bh,phc,bpsc->bps", hn, np.asarray(bilin_w, np.float32), cand)
    return out.astype(np.float32)


# revision 6
# speedup vs baseline: 1.2909x; 1.2909x over previous
"""CPC_Net forward pass: StagerNet embeddings on 8 NeuronCores (Bass),
GRU + bilinear scoring on host.

Device kernel layout: activations live as [(b4, c, o) partitions, position free].
conv2: block-diag over b-quads, K=(kp16,b4,ci2)=128, 4 accumulated tap-chunks,
       M=(b4,c,o)=128 Toeplitz weight columns (w1 folded in).
conv3: block-diag over s=(b4,c) blocks, 50 accumulated taps, tap = rhs free-offset.
BN: relu+sum via ACT accum_out, sumsq via DVE scalar_tensor_tensor accum,
    per-channel aggregation via mask matmuls, affine applied post-pool (g>0).
"""
import os
import sys

sys.path.insert(0, "/opt/trn_rl_repo")

import numpy as np

EPS = 1e-5
B, Np, Nb, T, C, H = 16, 16, 10, 3000, 2, 100
O = 16
L1 = T - 49            # 2951 = 13*227
G1 = L1 // 13          # 227
L2 = G1 - 49           # 178
G2 = 13
TP = 3072              # padded input length
NQ = 4                 # b-quads per window
KPC = 16               # kp positions per conv2 tap-chunk
NT2 = 4                # conv2 tap chunks
PC1 = 1024             # conv2 psum/relu chunk
W_PER_CORE = 24
N_CORES = 8

LAST_EXEC_NS = 0


# ---------------------------------------------------------------------------
# toolchain patches: this sandbox's walrus accepts at most ONE sync wait per
# instruction. Split multi-wait instructions at BIR-JSON level; split the
# Tile tail drain's waits across NoOps.
def _install_patches():
    import json as _json
    import concourse.bass as _bass
    import concourse.tile as _tile
    from concourse.vector_clock import ScopedClock
    import bass_rust

    if getattr(_bass.Bass, "_cpc_patched", False):
        class _Holder:
            pass
    else:
        _orig = _bass.Bass.to_json_bytes

        def _split_multiwait_json(self):
            raw = _orig(self)
            data = _json.loads(raw)
            changed = False
            for fn in data.get("functions", []):
                for blk in fn.get("blocks", []):
                    insts = blk.get("instructions", [])
                    out = []
                    for inst in insts:
                        si = inst.get("sync_info")
                        waits = (si or {}).get("on_wait") or []
                        if len(waits) > 1:
                            changed = True
                            for k, wx in enumerate(waits[:-1]):
                                out.append({
                                    "debug": inst.get("debug", 0),
                                    "engine": inst["engine"],
                                    "ins": [], "outs": [],
                                    "name": f"{inst['name']}-sw{k}",
                                    "opcode": "NoOp",
                                    "sync_info": {"on_update": [], "on_wait": [wx]},
                                    "text_hint": "split_wait",
                                })
                            si["on_wait"] = waits[-1:]
                        out.append(inst)
                    if changed:
                        blk["instructions"] = out
            return _json.dumps(data).encode() if changed else raw

        _bass.Bass.to_json_bytes = _split_multiwait_json
        _bass.Bass._cpc_patched = True

    class PatchedTC(_tile.TileContext):
        def _drain_and_barrier(self, tick_clock, wait_clock):
            probe = self.nc.sync.nop(hint="drain_waits", nofuse=True)
            wait_clock.add_sem_waits(
                probe.ins, ScopedClock({None: tick_clock.global_clock})
            )
            si = probe.ins.sync_info
            waits = list(si.on_wait) if si is not None else []
            if len(waits) > 1:
                si.on_wait = waits[:1]
                for w in waits[1:]:
                    n2 = self.nc.sync.nop(hint="drain_waits2", nofuse=True)
                    s2 = n2.ins.sync_info
                    if s2 is None:
                        n2.ins.sync_info = bass_rust.SyncInfo(on_wait=[w], on_update=[])
                    else:
                        s2.on_wait = [w]
            self.nc.sync.drain()
            self.nc.all_engine_barrier()
            popped = self.nc._tile_sem_poison_stack.pop()
            assert popped is self._sem_poison
            self.nc.clear_and_free_semaphores(list(self.sems.allocated().values()))
            self.nc.all_engine_barrier()

    return PatchedTC


# ---------------------------------------------------------------------------
def _host_weights(w1, w2, w3):
    """Precompute device weight matrices. w1:(C,C) w2:(16,50) w3:(16,16,50)."""
    cw2 = np.zeros((NT2, 128, 128), np.float32)
    for t in range(NT2):
        for kp in range(KPC):
            r = t * KPC + kp
            if r >= 50:
                continue
            for b4 in range(4):
                p = kp * 8 + b4 * 2
                for ci in range(C):
                    for c in range(C):
                        m0 = b4 * 32 + c * 16
                        cw2[t, p + ci, m0:m0 + O] = w2[:, r] * w1[c, ci]
    cw3 = np.zeros((50, 128, 128), np.float32)
    for r in range(50):
        blk = w3[:, :, r].T
        for s in range(8):
            cw3[r, s * 16:(s + 1) * 16, s * 16:(s + 1) * 16] = blk
    mask = np.zeros((128, 16), np.float32)
    for p in range(128):
        mask[p, p % 16] = 1.0
    return cw2, cw3, mask, np.ascontiguousarray(mask.T)


def _build_kernel(W):
    import concourse.bass as bass
    import concourse.mybir as mybir

    PatchedTC = _install_patches()
    f32 = mybir.dt.float32
    bf16 = mybir.dt.bfloat16
    fp16 = mybir.dt.float16
    AF = mybir.ActivationFunctionType
    OP = mybir.AluOpType

    nc = bass.Bass(num_devices=N_CORES, debug=False)
    xw_d = nc.dram_tensor("xw", [W, B, C, TP], fp16, kind="ExternalInput")
    cw2_d = nc.dram_tensor("cw2", [NT2, 128, 128], fp16, kind="ExternalInput")
    cw3_d = nc.dram_tensor("cw3", [50, 128, 128], f32, kind="ExternalInput")
    mask_d = nc.dram_tensor("mask", [128, 16], f32, kind="ExternalInput")
    maskT_d = nc.dram_tensor("maskT", [16, 128], f32, kind="ExternalInput")
    bnp_d = nc.dram_tensor("bnp", [16, 4], f32, kind="ExternalInput")
    x4_d = nc.dram_tensor("x4", [W, NQ, 128, G2], f32, kind="ExternalOutput")

    N1 = float(B * L1 * C)
    N2 = float(B * L2 * C)

    with PatchedTC(nc) as tc:
        with (
            tc.tile_pool(name="const", bufs=1) as cp,
            tc.tile_pool(name="xin", bufs=3) as xp,
            tc.tile_pool(name="act", bufs=2) as ap,
            tc.tile_pool(name="scr", bufs=1) as scp,
            tc.tile_pool(name="zp", bufs=2) as zpp,
            tc.tile_pool(name="stat", bufs=2) as stp,
            tc.tile_pool(name="ps", bufs=2, space="PSUM") as psp,
            tc.tile_pool(name="ps3", bufs=2, space="PSUM") as ps3p,
            tc.tile_pool(name="pss", bufs=2, space="PSUM") as pssp,
        ):
            cw2_t = cp.tile([128, NT2 * 128], fp16)
            for t in range(NT2):
                nc.sync.dma_start(out=cw2_t[:, t * 128:(t + 1) * 128], in_=cw2_d[t])
            cw3_t = cp.tile([128, 50 * 128], f32)
            for r in range(50):
                nc.sync.dma_start(out=cw3_t[:, r * 128:(r + 1) * 128], in_=cw3_d[r])
            mask_t = cp.tile([128, 16], f32)
            nc.sync.dma_start(out=mask_t[:, :], in_=mask_d[:, :])
            maskT_t = cp.tile([16, 128], f32)
            nc.sync.dma_start(out=maskT_t[:, :], in_=maskT_d[:, :])
            bnp_t = cp.tile([16, 4], f32)
            nc.sync.dma_start(out=bnp_t[:, :], in_=bnp_d[:, :])

            for w in range(W):
                zp_tiles = []
                st1 = stp.tile([128, 24], f32, tag="st1")
                for q in range(NQ):
                    x2s = xp.tile([128, T], fp16, tag="x2s")
                    for kp in range(KPC):
                        nc.sync.dma_start(
                            out=x2s[kp * 8:(kp + 1) * 8, :],
                            in_=xw_d[w, q * 4:(q + 1) * 4, :, kp:kp + T].rearrange("b c t -> (b c) t"),
                        )
                    zr1 = ap.tile([128, L1], f32, tag="zr1")
                    scr = scp.tile([128, PC1], f32, tag="scr")
                    for pc in range(3):
                        p0 = pc * PC1
                        ncol = min(PC1, L1 - p0)
                        zc = psp.tile([128, PC1], f32, tag="zc")
                        for sub in range(0, ncol, 512):
                            nsub = min(512, ncol - sub)
                            for t in range(NT2):
                                nc.tensor.matmul(
                                    zc[:, sub:sub + nsub],
                                    cw2_t[:, t * 128:(t + 1) * 128],
                                    x2s[:, p0 + sub + t * KPC: p0 + sub + t * KPC + nsub],
                                    start=(t == 0), stop=(t == NT2 - 1),
                                )
                        nc.scalar.activation(
                            out=zr1[:, p0:p0 + ncol], in_=zc[:, :ncol], func=AF.Relu,
                            accum_out=st1[:, q * 3 + pc: q * 3 + pc + 1],
                        )
                        nc.vector.scalar_tensor_tensor(
                            out=scr[:, :ncol], in0=zr1[:, p0:p0 + ncol], scalar=1.0,
                            in1=zr1[:, p0:p0 + ncol], op0=OP.mult, op1=OP.mult,
                            accum_out=st1[:, 12 + q * 3 + pc: 12 + q * 3 + pc + 1],
                        )
                    zp1 = zpp.tile([128, G1], f32, tag=f"zp{q}")
                    nc.vector.tensor_reduce(
                        out=zp1[:, :], in_=zr1[:, :].rearrange("p (g j) -> p g j", j=13),
                        axis=mybir.AxisListType.X, op=OP.max,
                    )
                    zp_tiles.append(zp1)

                ms1 = pssp.tile([128, 32], f32, tag="msx")
                nc.tensor.matmul(ms1[0:16, 0:24], mask_t[:, :], st1[:, :], start=True, stop=True)
                s1 = stp.tile([16, 2], f32, tag="s1")
                nc.vector.tensor_reduce(out=s1[:, 0:1], in_=ms1[0:16, 0:12],
                                        axis=mybir.AxisListType.X, op=OP.add)
                nc.vector.tensor_reduce(out=s1[:, 1:2], in_=ms1[0:16, 12:24],
                                        axis=mybir.AxisListType.X, op=OP.add)
                fin1 = stp.tile([16, 6], f32, tag="fin1")
                nc.vector.tensor_scalar_mul(fin1[:, 0:1], s1[:, 0:1], 1.0 / N1)
                nc.vector.tensor_scalar_mul(fin1[:, 1:2], s1[:, 1:2], 1.0 / N1)
                nc.vector.tensor_mul(fin1[:, 2:3], fin1[:, 0:1], fin1[:, 0:1])
                nc.vector.tensor_sub(fin1[:, 2:3], fin1[:, 1:2], fin1[:, 2:3])
                nc.vector.tensor_scalar_add(fin1[:, 2:3], fin1[:, 2:3], EPS)
                nc.vector.reciprocal(fin1[:, 3:4], fin1[:, 2:3])
                nc.scalar.activation(out=fin1[:, 3:4], in_=fin1[:, 3:4], func=AF.Sqrt)
                nc.vector.tensor_mul(fin1[:, 4:5], bnp_t[:, 0:1], fin1[:, 3:4])
                nc.vector.tensor_mul(fin1[:, 5:6], fin1[:, 0:1], fin1[:, 4:5])
                nc.vector.tensor_sub(fin1[:, 5:6], bnp_t[:, 1:2], fin1[:, 5:6])
                bc1 = pssp.tile([128, 32], f32, tag="msx")
                nc.tensor.matmul(bc1[:, 0:2], maskT_t[:, :], fin1[:, 4:6], start=True, stop=True)
                scb1 = stp.tile([128, 2], f32, tag="scb1")
                nc.vector.tensor_copy(scb1[:, :], bc1[:, 0:2])

                x3all = ap.tile([128, NQ * 232], f32, tag="x3all")
                for q in range(NQ):
                    nc.vector.tensor_scalar(
                        out=x3all[:, q * 232:q * 232 + G1], in0=zp_tiles[q][:, :],
                        scalar1=scb1[:, 0:1], scalar2=scb1[:, 1:2], op0=OP.mult, op1=OP.add,
                    )
                st2 = stp.tile([128, 8], f32, tag="st2")
                zr3_tiles = []
                for q in range(NQ):
                    c3 = ps3p.tile([128, L2], f32, tag="c3")
                    for r in range(50):
                        nc.tensor.matmul(
                            c3[:, :],
                            cw3_t[:, r * 128:(r + 1) * 128],
                            x3all[:, q * 232 + r: q * 232 + r + L2],
                            start=(r == 0), stop=(r == 49),
                        )
                    zr3 = ap.tile([128, L2], f32, tag=f"zr3_{q}")
                    nc.scalar.activation(
                        out=zr3[:, :], in_=c3[:, :], func=AF.Relu,
                        accum_out=st2[:, q: q + 1],
                    )
                    scr3 = scp.tile([128, PC1], f32, tag="scr")
                    nc.vector.scalar_tensor_tensor(
                        out=scr3[:, :L2], in0=zr3[:, :], scalar=1.0, in1=zr3[:, :],
                        op0=OP.mult, op1=OP.mult,
                        accum_out=st2[:, 4 + q: 5 + q],
                    )
                    zr3_tiles.append(zr3)
                ms2 = pssp.tile([128, 32], f32, tag="msx")
                nc.tensor.matmul(ms2[0:16, 0:8], mask_t[:, :], st2[:, :], start=True, stop=True)
                s2 = stp.tile([16, 2], f32, tag="s2")
                nc.vector.tensor_reduce(out=s2[:, 0:1], in_=ms2[0:16, 0:4],
                                        axis=mybir.AxisListType.X, op=OP.add)
                nc.vector.tensor_reduce(out=s2[:, 1:2], in_=ms2[0:16, 4:8],
                                        axis=mybir.AxisListType.X, op=OP.add)
                fin2 = stp.tile([16, 6], f32, tag="fin2")
                nc.vector.tensor_scalar_mul(fin2[:, 0:1], s2[:, 0:1], 1.0 / N2)
                nc.vector.tensor_scalar_mul(fin2[:, 1:2], s2[:, 1:2], 1.0 / N2)
                nc.vector.tensor_mul(fin2[:, 2:3], fin2[:, 0:1], fin2[:, 0:1])
                nc.vector.tensor_sub(fin2[:, 2:3], fin2[:, 1:2], fin2[:, 2:3])
                nc.vector.tensor_scalar_add(fin2[:, 2:3], fin2[:, 2:3], EPS)
                nc.vector.reciprocal(fin2[:, 3:4], fin2[:, 2:3])
                nc.scalar.activation(out=fin2[:, 3:4], in_=fin2[:, 3:4], func=AF.Sqrt)
                nc.vector.tensor_mul(fin2[:, 4:5], bnp_t[:, 2:3], fin2[:, 3:4])
                nc.vector.tensor_mul(fin2[:, 5:6], fin2[:, 0:1], fin2[:, 4:5])
                nc.vector.tensor_sub(fin2[:, 5:6], bnp_t[:, 3:4], fin2[:, 5:6])
                bc2 = pssp.tile([128, 32], f32, tag="msx")
                nc.tensor.matmul(bc2[:, 0:2], maskT_t[:, :], fin2[:, 4:6], start=True, stop=True)
                scb2 = stp.tile([128, 2], f32, tag="scb2")
                nc.vector.tensor_copy(scb2[:, :], bc2[:, 0:2])

                for q in range(NQ):
                    zp3 = zpp.tile([128, G2], f32, tag="zp3")
                    nc.vector.tensor_reduce(
                        out=zp3[:, :], in_=zr3_tiles[q][:, 0:169].rearrange("p (g j) -> p g j", j=13),
                        axis=mybir.AxisListType.X, op=OP.max,
                    )
                    x4q = zpp.tile([128, G2], f32, tag="x4q")
                    nc.vector.tensor_scalar(
                        out=x4q[:, :], in0=zp3[:, :],
                        scalar1=scb2[:, 0:1], scalar2=scb2[:, 1:2], op0=OP.mult, op1=OP.add,
                    )
                    nc.sync.dma_start(out=x4_d[w, q], in_=x4q[:, :])
    return nc


def _device_stagenet(Xc, Xp, Xb, w1, w2, w3, g1, be1, g2, be2):
    """-> (192, B, 416) pooled features in window order [Xb(160), Xc(16), Xp(16)]."""
    global LAST_EXEC_NS
    from concourse import bass_utils

    # pack windows: (192, B, C, TP)
    xb = Xb.transpose(1, 2, 0, 4, 3).reshape(Np * Nb, B, C, T)
    xc = Xc.transpose(1, 0, 3, 2)
    xpp = Xp.transpose(1, 0, 3, 2)
    allw = np.concatenate([xb, xc, xpp], axis=0)
    xw_all = np.zeros((192, B, C, TP), np.float16)
    xw_all[:, :, :, :T] = allw

    cw2, cw3, mask, maskT = _host_weights(w1, w2, w3)
    bnp = np.stack([g1, be1, g2, be2], axis=1).astype(np.float32)

    W = W_PER_CORE
    nc = _build_kernel(W)
    in_maps = []
    for k in range(N_CORES):
        in_maps.append({
            "xw": np.ascontiguousarray(xw_all[k * W:(k + 1) * W]),
            "cw2": cw2.astype(np.float16), "cw3": cw3, "mask": mask, "maskT": maskT, "bnp": bnp,
        })
    import time as _time
    _t0 = _time.time()
    res = bass_utils.run_bass_kernel_spmd(nc, in_maps, core_ids=list(range(N_CORES)))
    LAST_EXEC_NS = int((_time.time() - _t0) * 1e9)

    feats = np.empty((192, B, 16 * G2 * C), np.float32)
    for k in range(N_CORES):
        x4 = res.results[k]["x4"]                    # (W, 4, 128, G2)
        x = x4.reshape(W, 4, 4, 2, 16, G2)           # (w, quad, b4, c, o3, g)
        f = x.transpose(0, 1, 2, 4, 5, 3)            # (w, quad, b4, o3, g, c)
        feats[k * W:(k + 1) * W] = f.reshape(W, 16, 16 * G2 * 2)
    return feats


# ---------------------------------------------------------------------------
# host fallback stagenet (numpy), from the original baseline
def _conv_tap(x, K):
    S, L = x.shape
    O_, R = K.shape
    Lo = L - R + 1
    out = np.empty((S, O_, Lo), np.float32)
    sv = np.lib.stride_tricks.as_strided(
        x, (S, Lo, R), (x.strides[0], x.strides[1], x.strides[1])
    )
    cs = max(1, S // 8)
    for i in range(0, S, cs):
        out[i:i + cs] = np.tensordot(sv[i:i + cs], K, axes=([2], [1])).transpose(0, 2, 1)
    return out


def _stagenet_batch_host(x, p):
    N = x.shape[0]
    y = x @ p["w1"].T + p["b1"]
    seq = y.transpose(0, 1, 3, 2).reshape(N * B * C, T).astype(np.float32)
    z = _conv_tap(seq, p["w2"])
    z = z.reshape(N, B, C, 16, L1).transpose(0, 1, 3, 4, 2) + p["b2"][None, None, :, None, None]
    z = np.maximum(z, 0.0)
    m = z.mean(axis=(1, 3, 4), keepdims=True)
    v = z.var(axis=(1, 3, 4), keepdims=True)
    z = (z - m) / np.sqrt(v + EPS) * p["g1"][None, None, :, None, None] + p["be1"][None, None, :, None, None]
    z = z.reshape(N, B, 16, G1, 13, C).max(axis=4)
    seq3 = z.transpose(0, 1, 4, 2, 3).reshape(N * B * C, 16, G1)
    sv = np.lib.stride_tricks.as_strided(
        seq3, (N * B * C, L2, 16, 50),
        (seq3.strides[0], seq3.strides[2], seq3.strides[1], seq3.strides[2]),
    )
    W3 = p["w3"].reshape(16, 16 * 50)
    z3 = np.empty((N * B * C, L2, 16), np.float32)
    cs = max(1, (N * B * C) // 8)
    for i in range(0, N * B * C, cs):
        z3[i:i + cs] = (sv[i:i + cs].reshape(-1, 16 * 50) @ W3.T).reshape(-1, L2, 16)
    z3 = z3.reshape(N, B, C, L2, 16).transpose(0, 1, 4, 3, 2) + p["b3"][None, None, :, None, None]
    z3 = np.maximum(z3, 0.0)
    m = z3.mean(axis=(1, 3, 4), keepdims=True)
    v = z3.var(axis=(1, 3, 4), keepdims=True)
    z3 = (z3 - m) / np.sqrt(v + EPS) * p["g2"][None, None, :, None, None] + p["be2"][None, None, :, None, None]
    z3 = z3[:, :, :, :169, :].reshape(N, B, 16, 13, 13, C).max(axis=4)
    return z3.reshape(N, B, 16 * 13 * C)


def _gru_last(xs, wih, whh, bih, bhh):
    h = np.zeros((xs.shape[0], whh.shape[1]), np.float32)
    for t in range(xs.shape[1]):
        gi = xs[:, t] @ wih.T + bih
        gh = h @ whh.T + bhh
        ir, iz, inn = np.split(gi, 3, axis=-1)
        hr, hz, hnn = np.split(gh, 3, axis=-1)
        r = 1.0 / (1.0 + np.exp(-(ir + hr)))
        z = 1.0 / (1.0 + np.exp(-(iz + hz)))
        n = np.tanh(inn + r * hnn)
        h = (1.0 - z) * n + z * h
    return h


def kernel(Xc, Xp, Xb, sn_w1, sn_b1, sn_w2, sn_b2, sn_g1, sn_be1, sn_w3, sn_b3,
           sn_g2, sn_be2, sn_wl, sn_bl, gru_wih, gru_whh, gru_bih, gru_bhh, bilin_w):
    Xc = np.asarray(Xc, np.float32)
    Xp = np.asarray(Xp, np.float32)
    Xb = np.asarray(Xb, np.float32)
    w1 = np.asarray(sn_w1, np.float32)
    w2 = np.asarray(sn_w2, np.float32).reshape(16, 50)
    w3 = np.asarray(sn_w3, np.float32)[:, :, :, 0]
    g1 = np.asarray(sn_g1, np.float32); be1 = np.asarray(sn_be1, np.float32)
    g2 = np.asarray(sn_g2, np.float32); be2 = np.asarray(sn_be2, np.float32)
    wl = np.asarray(sn_wl, np.float32); bl = np.asarray(sn_bl, np.float32)
    b1 = np.asarray(sn_b1, np.float32); b2 = np.asarray(sn_b2, np.float32)
    b3 = np.asarray(sn_b3, np.float32)

    use_device = (
        abs(b1).max() == 0 and abs(b2).max() == 0 and abs(b3).max() == 0
        and (g1 > 0).all() and (g2 > 0).all()
    )
    feats = None
    if use_device:
        try:
            feats = _device_stagenet(Xc, Xp, Xb, w1, w2, w3, g1, be1, g2, be2)
        except Exception as e:
            print(f"kernel: device stagenet failed ({type(e).__name__}: {e}); host fallback")
            feats = None
    if feats is None:
        p = {"w1": w1, "b1": b1, "w2": w2, "b2": b2, "g1": g1, "be1": be1,
             "w3": w3, "b3": b3, "g2": g2, "be2": be2}
        xb_in = Xb.transpose(1, 2, 0, 3, 4).reshape(Np * Nb, B, T, C)
        xc_in = Xc.transpose(1, 0, 2, 3)
        xp_in = Xp.transpose(1, 0, 2, 3)
        feats = np.concatenate([
            _stagenet_batch_host(xb_in, p),
            _stagenet_batch_host(xc_in, p),
            _stagenet_batch_host(xp_in, p),
        ], axis=0)

    emb = feats @ wl.T + bl                       # (192, B, 100)
    xb_e = emb[:160].reshape(Np, Nb, B, H).transpose(2, 0, 1, 3)   # (B,Np,Nb,H)
    xc_e = emb[160:176].transpose(1, 0, 2)        # (B,16,H)
    xp_e = emb[176:192].transpose(1, 0, 2)        # (B,Np,H)

    hn = _gru_last(xc_e, np.asarray(gru_wih, np.float32), np.asarray(gru_whh, np.float32),
                   np.asarray(gru_bih, np.float32), np.asarray(gru_bhh, np.float32))
    cand = np.concatenate([xp_e[:, :, None, :], xb_e], axis=2)     # (B,Np,Nb+1,H)
    out = np.einsum("bh,phc,bpsc->bps", hn, np.asarray(bilin_w, np.float32), cand)
    return out.astype(np.float32)
